# revision 3
# baseline (speedup 1.0000x reference)
"""CIF layer (continuous integrate-and-fire) on 8 TRN2 NeuronCores.

Strategy: pure data parallel over batch (2 batches per core). The small
routing computation (sigmoid / cumsum / fire decisions, (B,S) = 16x4096)
mirrors the reference's exact jnp op sequence on the default backend so the
integer fire positions match the oracle bit-for-bit. All heavy work -- the
134 MB of x through the banded attention matmul and the output projection --
runs in a Bass/Tile kernel on the 8 cores.

Self-contained: hardcodes shapes from the problem spec.
"""
import os
import sys
import types
from contextlib import ExitStack

import numpy as np

S, B, C = 4096, 16, 512
T_MAX = 512
BETA = 1.0
N_CORES = 8
BPC = B // N_CORES          # batches per core
W_WIN = 256                 # target-window width
N_WIN = T_MAX // W_WIN      # 2 windows
NBLK = S // 128             # 32 source blocks per batch

MM_DT = "float32"           # matmul dtype: float32 | float32r


def _routing(x, mask, tl):
    """Mirror reference.py's alpha/cumsum/dest computation op-for-op (un-jitted,
    default jax backend) so fire positions are bit-identical to the oracle."""
    import jax
    import jax.numpy as jnp

    x = jnp.asarray(x)
    mask = jnp.asarray(mask)
    tl = jnp.asarray(tl)
    xb_last = jnp.transpose(x, (1, 0, 2))[..., -1]                # (B, S)
    a = jnp.where(mask, jnp.asarray(-10000.0, x.dtype), xb_last)
    alpha = jax.nn.sigmoid(a)
    alpha_sum = alpha.sum(1)
    tlf = tl.astype(x.dtype)
    alpha2 = alpha * BETA * tlf[:, None] / alpha_sum[:, None]
    csum = jnp.cumsum(alpha2, axis=-1)
    dest = jnp.concatenate(
        [jnp.zeros((x.shape[1], 1), jnp.int32),
         jnp.floor(csum / BETA + 1e-4).astype(jnp.int32)], axis=1)  # (B, S+1)
    fire_num = dest[:, 1:] - dest[:, :-1]
    extra = jnp.clip(fire_num - 1, 0).astype(x.dtype) * BETA
    right_w = jnp.where(fire_num > 0,
                        csum - dest[:, 1:].astype(x.dtype) * BETA, alpha2)
    left_w = alpha2 - right_w - extra
    return (np.asarray(dest), np.asarray(left_w), np.asarray(right_w),
            np.asarray(alpha_sum), int(np.asarray(fire_num).max()))


def _schedule(dest):
    """Union (over all batches) of source-block -> target-window pairs."""
    sched = {i: set() for i in range(N_WIN)}
    for b in range(B):
        for j in range(NBLK):
            lo = int(dest[b, 128 * j])
            hi = min(int(dest[b, 128 * j + 128]), T_MAX - 1)
            if lo > hi:
                continue
            for i in range(lo // W_WIN, hi // W_WIN + 1):
                if i < N_WIN:
                    sched[i].add(j)
    return {i: sorted(v) for i, v in sched.items()}


def _build_program(sched):
    import concourse.mybir as mybir
    import concourse.tile as tile
    from concourse import bacc

    f32 = mybir.dt.float32
    mm_dt = getattr(mybir.dt, MM_DT)
    is_eq = mybir.AluOpType.is_equal
    mult = mybir.AluOpType.mult

    nc = bacc.Bacc("TRN2", target_bir_lowering=False, debug=False,
                   num_devices=N_CORES)
    x_loc = nc.dram_tensor("x_loc", [S, BPC, C], f32, kind="ExternalInput").ap()
    wt = nc.dram_tensor("wt", [C, C], f32, kind="ExternalInput").ap()
    d0f = nc.dram_tensor("d0f", [BPC, 128, NBLK], f32, kind="ExternalInput").ap()
    d1f = nc.dram_tensor("d1f", [BPC, 128, NBLK], f32, kind="ExternalInput").ap()
    lwf = nc.dram_tensor("lwf", [BPC, 128, NBLK], f32, kind="ExternalInput").ap()
    rwf = nc.dram_tensor("rwf", [BPC, 128, NBLK], f32, kind="ExternalInput").ap()
    iot = nc.dram_tensor("iot", [N_WIN, 128, W_WIN], f32, kind="ExternalInput").ap()
    out_loc = nc.dram_tensor("out_loc", [T_MAX, BPC, C], f32,
                             kind="ExternalOutput").ap()

    with tile.TileContext(nc) as tc, ExitStack() as ctx:
        const_pool = ctx.enter_context(tc.tile_pool(name="const", bufs=1))
        xpool = ctx.enter_context(tc.tile_pool(name="x", bufs=6))
        rpool = ctx.enter_context(tc.tile_pool(name="routing", bufs=2))
        bpool = ctx.enter_context(tc.tile_pool(name="build", bufs=4))
        fpool = ctx.enter_context(tc.tile_pool(name="fsb", bufs=2))
        opool = ctx.enter_context(tc.tile_pool(name="osb", bufs=3))
        psum_f = ctx.enter_context(tc.tile_pool(name="pf", bufs=1, space="PSUM"))
        psum_o = ctx.enter_context(tc.tile_pool(name="po", bufs=2, space="PSUM"))

        wt_tiles = []
        for q in range(4):
            t = const_pool.tile([128, C], mm_dt, tag=f"wt{q}")
            nc.sync.dma_start(t[:], wt[128 * q:128 * (q + 1), :])
            wt_tiles.append(t)
        iot_tiles = []
        for i in range(N_WIN):
            t = const_pool.tile([128, W_WIN], f32, tag=f"iot{i}")
            nc.sync.dma_start(t[:], iot[i])
            iot_tiles.append(t)

        for b in range(BPC):
            d0t = rpool.tile([128, NBLK], f32, tag="d0")
            nc.sync.dma_start(d0t[:], d0f[b])
            d1t = rpool.tile([128, NBLK], f32, tag="d1")
            nc.sync.dma_start(d1t[:], d1f[b])
            lwt = rpool.tile([128, NBLK], f32, tag="lw")
            nc.sync.dma_start(lwt[:], lwf[b])
            rwt = rpool.tile([128, NBLK], f32, tag="rw")
            nc.sync.dma_start(rwt[:], rwf[b])

            for i in range(N_WIN):
                js = sched[i]
                pf = None
                if js:
                    pf = [psum_f.tile([128, W_WIN], f32, tag=f"pf{q}",
                                      name=f"pf{q}_{b}_{i}")
                          for q in range(4)]
                for idx, j in enumerate(js):
                    xb = xpool.tile([128, C - 1], mm_dt, tag="xb")
                    nc.sync.dma_start(xb[:], x_loc[128 * j:128 * j + 128, b, 0:C - 1])
                    t1 = bpool.tile([128, W_WIN], mm_dt, tag="t1")
                    nc.vector.tensor_scalar(
                        out=t1[:], in0=iot_tiles[i][:],
                        scalar1=d0t[:, j:j + 1], scalar2=lwt[:, j:j + 1],
                        op0=is_eq, op1=mult)
                    t2 = bpool.tile([128, W_WIN], mm_dt, tag="t2")
                    nc.vector.tensor_scalar(
                        out=t2[:], in0=iot_tiles[i][:],
                        scalar1=d1t[:, j:j + 1], scalar2=rwt[:, j:j + 1],
                        op0=is_eq, op1=mult)
                    at = bpool.tile([128, W_WIN], mm_dt, tag="at")
                    nc.vector.tensor_add(at[:], t1[:], t2[:])
                    first, last = idx == 0, idx == len(js) - 1
                    for q in range(4):
                        m = 128 if q < 3 else C - 1 - 384
                        nc.tensor.matmul(pf[q][:m, :], xb[:, 128 * q:128 * q + m],
                                         at[:], start=first, stop=last)
                fsb = []
                for q in range(4):
                    m = 128 if q < 3 else C - 1 - 384
                    t = fpool.tile([128, W_WIN], mm_dt, tag=f"fsb{q}",
                                   name=f"fsb{q}_{b}_{i}")
                    if js:
                        nc.any.tensor_copy(t[:m], pf[q][:m])
                    else:
                        nc.vector.memset(t[:m], 0.0)
                    fsb.append(t)
                for h in range(W_WIN // 128):
                    po = psum_o.tile([128, C], f32, tag="po")
                    for q in range(4):
                        m = 128 if q < 3 else C - 1 - 384
                        nc.tensor.matmul(
                            po[:], fsb[q][:m, 128 * h:128 * h + 128],
                            wt_tiles[q][:m, :], start=(q == 0), stop=(q == 3))
                    ot = opool.tile([128, C], f32, tag="ot")
                    nc.any.tensor_copy(ot[:], po[:])
                    t0 = W_WIN * i + 128 * h
                    nc.sync.dma_start(out_loc[t0:t0 + 128, b, :], ot[:])
    nc.compile()
    return nc


def _make_in_maps(x, W_out, dest, left_w, right_w):
    d0 = dest[:, :-1].astype(np.float32)                    # (B, S)
    d1 = dest[:, 1:].astype(np.float32)
    lw = left_w.astype(np.float32)
    rw = right_w.astype(np.float32)

    def blk(a, b0):   # (BPC, S) -> (BPC, 128, NBLK) with [b, p, j] = a[b, 128j+p]
        return np.ascontiguousarray(
            a[b0:b0 + BPC].reshape(BPC, NBLK, 128).transpose(0, 2, 1))

    wt = np.zeros((C, C), np.float32)
    wt[:C - 1, :] = W_out.T
    iota = np.broadcast_to(
        np.arange(T_MAX, dtype=np.float32).reshape(N_WIN, 1, W_WIN),
        (N_WIN, 128, W_WIN))
    iota = np.ascontiguousarray(iota)

    in_maps = []
    for k in range(N_CORES):
        b0 = k * BPC
        in_maps.append({
            "x_loc": np.ascontiguousarray(x[:, b0:b0 + BPC, :]),
            "wt": wt,
            "d0f": blk(d0, b0),
            "d1f": blk(d1, b0),
            "lwf": blk(lw, b0),
            "rwf": blk(rw, b0),
            "iot": iota,
        })
    return in_maps


def _ensure_ntff_hook():
    """Register the axon NTFF profiling hook if the container's antenv stub
    lacks it (needed only when tracing is requested, e.g. BASS_TRACE=1)."""
    try:
        from antenv.axon_hooks import get_axon_ntff_profile_hook  # noqa: F401
        return
    except ImportError:
        pass
    try:
        import antenv
        from trn_agent_boot.trn_boot import _ntff_profile_via_ctypes
        mod = types.ModuleType("antenv.axon_hooks")
        _hook = [_ntff_profile_via_ctypes("/opt/axon/libaxon_pjrt.so")]

        def set_axon_ntff_profile_hook(h):
            _hook[0] = h

        def get_axon_ntff_profile_hook():
            return _hook[0]

        mod.set_axon_ntff_profile_hook = set_axon_ntff_profile_hook
        mod.get_axon_ntff_profile_hook = get_axon_ntff_profile_hook
        sys.modules["antenv.axon_hooks"] = mod
        antenv.axon_hooks = mod
    except Exception:
        pass


def _host_fallback(x, encoder_padding_mask, target_lengths, W_out, b_out):
    """Full jnp mirror of the reference (only for fire_num > 1, which cannot
    happen for the spec'd input distribution)."""
    import jax.numpy as jnp
    xb = jnp.transpose(jnp.asarray(x), (1, 0, 2))
    mask = jnp.asarray(encoder_padding_mask)
    tl = jnp.asarray(target_lengths)
    Bn, Sn, Cn = xb.shape
    import jax
    a = jnp.where(mask, jnp.asarray(-10000.0, xb.dtype), xb[..., -1])
    alpha = jax.nn.sigmoid(a)
    alpha_sum = alpha.sum(1)
    tlf = tl.astype(xb.dtype)
    alpha = alpha * BETA * tlf[:, None] / alpha_sum[:, None]
    csum = jnp.cumsum(alpha, axis=-1)
    dest = jnp.concatenate(
        [jnp.zeros((Bn, 1), jnp.int32),
         jnp.floor(csum / BETA + 1e-4).astype(jnp.int32)], axis=1)
    fire_num = dest[:, 1:] - dest[:, :-1]
    extra = jnp.clip(fire_num - 1, 0).astype(xb.dtype) * BETA
    right_mask = fire_num > 0
    right_w = jnp.where(right_mask, csum - dest[:, 1:].astype(xb.dtype) * BETA, alpha)
    left_w = alpha - right_w - extra
    bi = jnp.arange(Bn)[:, None]
    si = jnp.arange(Sn)[None, :]
    attn = jnp.zeros((Bn, T_MAX + 1, Sn), xb.dtype)
    attn = attn.at[bi, dest[:, 1:], si].add(right_w)
    attn = attn.at[bi, dest[:, :-1], si].add(left_w)
    eidx = jnp.minimum(dest[:, :-1] + 1, T_MAX - 1)
    attn = attn.at[bi, eidx, si].add(jnp.minimum(extra, BETA))
    n_extra = jnp.clip(fire_num - 2, 0)
    rows = jnp.arange(T_MAX + 1)[None, :, None]
    start = (eidx + 1)[:, None, :]
    fill = (rows >= start) & (rows < start + n_extra[:, None, :])
    attn = jnp.where(fill, jnp.asarray(BETA, xb.dtype), attn)
    attn = attn[:, :T_MAX, :]
    feats = jnp.einsum('bts,bsc->btc', attn, xb[..., :-1])
    out = feats @ jnp.asarray(W_out).T + jnp.asarray(b_out)
    return np.asarray(jnp.transpose(out, (1, 0, 2))), np.asarray(alpha_sum)


def kernel(x, encoder_padding_mask, target_lengths, W_out, b_out,
           _trace=False, _nc_cache={}):
    dest, left_w, right_w, alpha_sum, max_fire = _routing(
        x, encoder_padding_mask, target_lengths)
    if max_fire > 1:
        return _host_fallback(x, encoder_padding_mask, target_lengths,
                              W_out, b_out)

    x_np = np.asarray(x, dtype=np.float32)
    W_np = np.asarray(W_out, dtype=np.float32)
    b_np = np.asarray(b_out, dtype=np.float32)

    sched = _schedule(dest)
    in_maps = _make_in_maps(x_np, W_np, dest, left_w, right_w)

    key = (tuple(tuple(v) for v in sched.values()), MM_DT)
    nc = _nc_cache.get(key)
    if nc is None:
        nc = _build_program(sched)
        _nc_cache[key] = nc

    if _trace or os.environ.get("BASS_TRACE"):
        _ensure_ntff_hook()
    from concourse.bass_utils import run_bass_kernel_spmd
    res = run_bass_kernel_spmd(nc, in_maps, core_ids=list(range(N_CORES)),
                               trace=_trace)

    out = np.empty((T_MAX, B, C), np.float32)
    for k in range(N_CORES):
        out[:, k * BPC:(k + 1) * BPC, :] = res.results[k]["out_loc"]
    out += b_np[None, None, :]
    if _trace:
        kernel._last_results = res
    return out, alpha_sum


# revision 9
# speedup vs baseline: 1.6423x; 1.6423x over previous
"""CIF layer (continuous integrate-and-fire) on 8 TRN2 NeuronCores.

Strategy: pure data parallel over batch (2 batches per core). The small
routing computation (sigmoid / cumsum / fire decisions, (B,S) = 16x4096)
mirrors the reference's exact jnp op sequence on the default backend so the
integer fire positions match the oracle bit-for-bit. All heavy work -- the
134 MB of x through the banded attention matmul and the output projection --
runs in a Bass/Tile kernel on the 8 cores.

Self-contained: hardcodes shapes from the problem spec.
"""
import os
import sys
import types
from contextlib import ExitStack

import numpy as np

S, B, C = 4096, 16, 512
T_MAX = 512
BETA = 1.0
N_CORES = 8
BPC = B // N_CORES          # batches per core
W_WIN = 256                 # target-window width
N_WIN = T_MAX // W_WIN      # 2 windows
NBLK = S // 128             # 32 source blocks per batch

MM_DT = os.environ.get("CIF_MM_DT", "float32r")  # matmul dtype: float32 | float32r


def _routing(x, mask, tl):
    """Mirror reference.py's alpha/cumsum/dest computation op-for-op (un-jitted,
    default jax backend) so fire positions are bit-identical to the oracle."""
    import jax
    import jax.numpy as jnp

    x = jnp.asarray(x)
    mask = jnp.asarray(mask)
    tl = jnp.asarray(tl)
    xb_last = jnp.transpose(x, (1, 0, 2))[..., -1]                # (B, S)
    a = jnp.where(mask, jnp.asarray(-10000.0, x.dtype), xb_last)
    alpha = jax.nn.sigmoid(a)
    alpha_sum = alpha.sum(1)
    tlf = tl.astype(x.dtype)
    alpha2 = alpha * BETA * tlf[:, None] / alpha_sum[:, None]
    csum = jnp.cumsum(alpha2, axis=-1)
    dest = jnp.concatenate(
        [jnp.zeros((x.shape[1], 1), jnp.int32),
         jnp.floor(csum / BETA + 1e-4).astype(jnp.int32)], axis=1)  # (B, S+1)
    fire_num = dest[:, 1:] - dest[:, :-1]
    extra = jnp.clip(fire_num - 1, 0).astype(x.dtype) * BETA
    right_w = jnp.where(fire_num > 0,
                        csum - dest[:, 1:].astype(x.dtype) * BETA, alpha2)
    left_w = alpha2 - right_w - extra
    return (np.asarray(dest), np.asarray(left_w), np.asarray(right_w),
            np.asarray(alpha_sum), int(np.asarray(fire_num).max()))


def _schedule(dest):
    """Union (over all batches) of source-block -> target-window pairs."""
    sched = {i: set() for i in range(N_WIN)}
    for b in range(B):
        for j in range(NBLK):
            lo = int(dest[b, 128 * j])
            hi = min(int(dest[b, 128 * j + 128]), T_MAX - 1)
            if lo > hi:
                continue
            for i in range(lo // W_WIN, hi // W_WIN + 1):
                if i < N_WIN:
                    sched[i].add(j)
    return {i: sorted(v) for i, v in sched.items()}


def _build_program(sched):
    import concourse.mybir as mybir
    import concourse.tile as tile
    from concourse import bacc

    f32 = mybir.dt.float32
    mm_dt = getattr(mybir.dt, MM_DT)
    is_eq = mybir.AluOpType.is_equal
    mult = mybir.AluOpType.mult

    def mm(ap):
        return ap

    nc = bacc.Bacc("TRN2", target_bir_lowering=False, debug=False,
                   num_devices=N_CORES)
    x_loc = nc.dram_tensor("x_loc", [S, BPC, C], mm_dt, kind="ExternalInput").ap()
    wt = nc.dram_tensor("wt", [C, C], mm_dt, kind="ExternalInput").ap()
    d0f = nc.dram_tensor("d0f", [BPC, 128, NBLK], f32, kind="ExternalInput").ap()
    d1f = nc.dram_tensor("d1f", [BPC, 128, NBLK], f32, kind="ExternalInput").ap()
    lwf = nc.dram_tensor("lwf", [BPC, 128, NBLK], f32, kind="ExternalInput").ap()
    rwf = nc.dram_tensor("rwf", [BPC, 128, NBLK], f32, kind="ExternalInput").ap()
    iot = nc.dram_tensor("iot", [N_WIN, 128, W_WIN], f32, kind="ExternalInput").ap()
    out_loc = nc.dram_tensor("out_loc", [T_MAX, BPC, C], f32,
                             kind="ExternalOutput").ap()

    with tile.TileContext(nc) as tc, ExitStack() as ctx:
        const_pool = ctx.enter_context(tc.tile_pool(name="const", bufs=1))
        xpool = ctx.enter_context(tc.tile_pool(name="x", bufs=6))
        rpool = ctx.enter_context(tc.tile_pool(name="routing", bufs=2))
        bpool = ctx.enter_context(tc.tile_pool(name="build", bufs=4))
        fpool = ctx.enter_context(tc.tile_pool(name="fsb", bufs=2))
        opool = ctx.enter_context(tc.tile_pool(name="osb", bufs=3))
        psum_f = ctx.enter_context(tc.tile_pool(name="pf", bufs=1, space="PSUM"))
        psum_o = ctx.enter_context(tc.tile_pool(name="po", bufs=2, space="PSUM"))

        wt_tiles = []
        for q in range(4):
            t = const_pool.tile([128, C], mm_dt, tag=f"wt{q}")
            nc.sync.dma_start(t[:], wt[128 * q:128 * (q + 1), :])
            wt_tiles.append(t)
        iot_tiles = []
        for i in range(N_WIN):
            t = const_pool.tile([128, W_WIN], f32, tag=f"iot{i}")
            nc.sync.dma_start(t[:], iot[i])
            iot_tiles.append(t)

        for b in range(BPC):
            d0t = rpool.tile([128, NBLK], f32, tag="d0")
            nc.sync.dma_start(d0t[:], d0f[b])
            d1t = rpool.tile([128, NBLK], f32, tag="d1")
            nc.sync.dma_start(d1t[:], d1f[b])
            lwt = rpool.tile([128, NBLK], f32, tag="lw")
            nc.sync.dma_start(lwt[:], lwf[b])
            rwt = rpool.tile([128, NBLK], f32, tag="rw")
            nc.sync.dma_start(rwt[:], rwf[b])

            for i in range(N_WIN):
                js = sched[i]
                pf = None
                if js:
                    pf = [psum_f.tile([128, W_WIN], f32, tag=f"pf{q}",
                                      name=f"pf{q}_{b}_{i}")
                          for q in range(4)]
                for idx, j in enumerate(js):
                    xb = xpool.tile([128, C - 1], mm_dt, tag="xb")
                    nc.sync.dma_start(xb[:], x_loc[128 * j:128 * j + 128, b, 0:C - 1])
                    t1 = bpool.tile([128, W_WIN], mm_dt, tag="t1")
                    nc.vector.tensor_scalar(
                        out=t1[:], in0=iot_tiles[i][:],
                        scalar1=d0t[:, j:j + 1], scalar2=lwt[:, j:j + 1],
                        op0=is_eq, op1=mult)
                    t2 = bpool.tile([128, W_WIN], mm_dt, tag="t2")
                    nc.vector.tensor_scalar(
                        out=t2[:], in0=iot_tiles[i][:],
                        scalar1=d1t[:, j:j + 1], scalar2=rwt[:, j:j + 1],
                        op0=is_eq, op1=mult)
                    at = bpool.tile([128, W_WIN], mm_dt, tag="at")
                    nc.vector.tensor_add(at[:], t1[:], t2[:])
                    first, last = idx == 0, idx == len(js) - 1
                    for q in range(4):
                        m = 128 if q < 3 else C - 1 - 384
                        nc.tensor.matmul(pf[q][:m, :],
                                         mm(xb[:, 128 * q:128 * q + m]),
                                         mm(at[:]), start=first, stop=last)
                fsb = []
                for q in range(4):
                    m = 128 if q < 3 else C - 1 - 384
                    t = fpool.tile([128, W_WIN], mm_dt, tag=f"fsb{q}",
                                   name=f"fsb{q}_{b}_{i}")
                    if js:
                        nc.any.tensor_copy(t[:m], pf[q][:m])
                    else:
                        nc.vector.memset(t[:m], 0.0)
                    fsb.append(t)
                for h in range(W_WIN // 128):
                    po = psum_o.tile([128, C], f32, tag="po")
                    for q in range(4):
                        m = 128 if q < 3 else C - 1 - 384
                        nc.tensor.matmul(
                            po[:], mm(fsb[q][:m, 128 * h:128 * h + 128]),
                            mm(wt_tiles[q][:m, :]), start=(q == 0), stop=(q == 3))
                    ot = opool.tile([128, C], f32, tag="ot")
                    nc.any.tensor_copy(ot[:], po[:])
                    t0 = W_WIN * i + 128 * h
                    nc.sync.dma_start(out_loc[t0:t0 + 128, b, :], ot[:])
    nc.compile()
    return nc


def _make_in_maps(x, W_out, dest, left_w, right_w):
    d0 = dest[:, :-1].astype(np.float32)                    # (B, S)
    d1 = dest[:, 1:].astype(np.float32)
    lw = left_w.astype(np.float32)
    rw = right_w.astype(np.float32)

    def blk(a, b0):   # (BPC, S) -> (BPC, 128, NBLK) with [b, p, j] = a[b, 128j+p]
        return np.ascontiguousarray(
            a[b0:b0 + BPC].reshape(BPC, NBLK, 128).transpose(0, 2, 1))

    wt = np.zeros((C, C), np.float32)
    wt[:C - 1, :] = W_out.T
    iota = np.broadcast_to(
        np.arange(T_MAX, dtype=np.float32).reshape(N_WIN, 1, W_WIN),
        (N_WIN, 128, W_WIN))
    iota = np.ascontiguousarray(iota)

    in_maps = []
    for k in range(N_CORES):
        b0 = k * BPC
        in_maps.append({
            "x_loc": np.ascontiguousarray(x[:, b0:b0 + BPC, :]),
            "wt": wt,
            "d0f": blk(d0, b0),
            "d1f": blk(d1, b0),
            "lwf": blk(lw, b0),
            "rwf": blk(rw, b0),
            "iot": iota,
        })
    return in_maps


def _ensure_ntff_hook():
    """Register the axon NTFF profiling hook if the container's antenv stub
    lacks it (needed only when tracing is requested, e.g. BASS_TRACE=1)."""
    try:
        from antenv.axon_hooks import get_axon_ntff_profile_hook  # noqa: F401
        return
    except ImportError:
        pass
    try:
        import antenv
        from trn_agent_boot.trn_boot import _ntff_profile_via_ctypes
        mod = types.ModuleType("antenv.axon_hooks")
        _hook = [_ntff_profile_via_ctypes("/opt/axon/libaxon_pjrt.so")]

        def set_axon_ntff_profile_hook(h):
            _hook[0] = h

        def get_axon_ntff_profile_hook():
            return _hook[0]

        mod.set_axon_ntff_profile_hook = set_axon_ntff_profile_hook
        mod.get_axon_ntff_profile_hook = get_axon_ntff_profile_hook
        sys.modules["antenv.axon_hooks"] = mod
        antenv.axon_hooks = mod
    except Exception:
        pass


def _host_fallback(x, encoder_padding_mask, target_lengths, W_out, b_out):
    """Full jnp mirror of the reference (only for fire_num > 1, which cannot
    happen for the spec'd input distribution)."""
    import jax.numpy as jnp
    xb = jnp.transpose(jnp.asarray(x), (1, 0, 2))
    mask = jnp.asarray(encoder_padding_mask)
    tl = jnp.asarray(target_lengths)
    Bn, Sn, Cn = xb.shape
    import jax
    a = jnp.where(mask, jnp.asarray(-10000.0, xb.dtype), xb[..., -1])
    alpha = jax.nn.sigmoid(a)
    alpha_sum = alpha.sum(1)
    tlf = tl.astype(xb.dtype)
    alpha = alpha * BETA * tlf[:, None] / alpha_sum[:, None]
    csum = jnp.cumsum(alpha, axis=-1)
    dest = jnp.concatenate(
        [jnp.zeros((Bn, 1), jnp.int32),
         jnp.floor(csum / BETA + 1e-4).astype(jnp.int32)], axis=1)
    fire_num = dest[:, 1:] - dest[:, :-1]
    extra = jnp.clip(fire_num - 1, 0).astype(xb.dtype) * BETA
    right_mask = fire_num > 0
    right_w = jnp.where(right_mask, csum - dest[:, 1:].astype(xb.dtype) * BETA, alpha)
    left_w = alpha - right_w - extra
    bi = jnp.arange(Bn)[:, None]
    si = jnp.arange(Sn)[None, :]
    attn = jnp.zeros((Bn, T_MAX + 1, Sn), xb.dtype)
    attn = attn.at[bi, dest[:, 1:], si].add(right_w)
    attn = attn.at[bi, dest[:, :-1], si].add(left_w)
    eidx = jnp.minimum(dest[:, :-1] + 1, T_MAX - 1)
    attn = attn.at[bi, eidx, si].add(jnp.minimum(extra, BETA))
    n_extra = jnp.clip(fire_num - 2, 0)
    rows = jnp.arange(T_MAX + 1)[None, :, None]
    start = (eidx + 1)[:, None, :]
    fill = (rows >= start) & (rows < start + n_extra[:, None, :])
    attn = jnp.where(fill, jnp.asarray(BETA, xb.dtype), attn)
    attn = attn[:, :T_MAX, :]
    feats = jnp.einsum('bts,bsc->btc', attn, xb[..., :-1])
    out = feats @ jnp.asarray(W_out).T + jnp.asarray(b_out)
    return np.asarray(jnp.transpose(out, (1, 0, 2))), np.asarray(alpha_sum)


def kernel(x, encoder_padding_mask, target_lengths, W_out, b_out,
           _trace=False, _nc_cache={}):
    dest, left_w, right_w, alpha_sum, max_fire = _routing(
        x, encoder_padding_mask, target_lengths)
    if max_fire > 1:
        return _host_fallback(x, encoder_padding_mask, target_lengths,
                              W_out, b_out)

    x_np = np.asarray(x, dtype=np.float32)
    W_np = np.asarray(W_out, dtype=np.float32)
    b_np = np.asarray(b_out, dtype=np.float32)

    sched = _schedule(dest)
    in_maps = _make_in_maps(x_np, W_np, dest, left_w, right_w)

    key = (tuple(tuple(v) for v in sched.values()), MM_DT)
    nc = _nc_cache.get(key)
    if nc is None:
        nc = _build_program(sched)
        _nc_cache[key] = nc

    if _trace or os.environ.get("BASS_TRACE"):
        _ensure_ntff_hook()
    from concourse.bass_utils import run_bass_kernel_spmd
    res = run_bass_kernel_spmd(nc, in_maps, core_ids=list(range(N_CORES)),
                               trace=_trace)

    out = np.empty((T_MAX, B, C), np.float32)
    for k in range(N_CORES):
        out[:, k * BPC:(k + 1) * BPC, :] = res.results[k]["out_loc"]
    out += b_np[None, None, :]
    if _trace:
        kernel._last_results = res
    return out, alpha_sum
